# revision 42
# baseline (speedup 1.0000x reference)
"""Causal self-attention (QKV proj + RoPE + causal SDPA + out proj) on 8 trn2 cores.

Sharding: tensor-parallel over heads. Each core owns 2 of 16 heads:
  - Wqkv column-split (the core's q/k/v head rows), Wproj row-split.
  - Each core computes a full-shape partial of the output projection;
    the 8 partials are summed (and transposed back) on the host.

Device-side layout: everything runs transposed (x^T fed as [C, B*T];
qkv^T = W @ x keeps head dims on partitions). v is transposed on-chip
with DMA xbar transposes.

Precision: query panels 512.. (pp>=1) run the QKV projection in
fp8-e4m3 DoubleRow (2 k-tiles per PE pass, 2x matmul rate); panel 0 of
each batch stays bf16 because short-context rows lack softmax error
attenuation. Attention itself stays bf16/f16.

PE offloads vs the all-bf16 version: rotate-half runs as SBUF->SBUF
DMA partition swaps (gpsimd queue) instead of PE permute-matmuls; the
softmax denominator broadcast matmul is deferred one job behind its
scalar copy so it never stalls the in-order PE queue.

Schedule: batch b's attention+outproj is woven with batch b+1's
projection so the tensor engine never starves on the exp (scalar
engine) dependency chain.
"""
import sys

sys.path.insert(0, "/opt/trn_rl_repo")

import numpy as np
import ml_dtypes

import concourse.bacc as bacc
import concourse.mybir as mybir
import concourse.tile as tile
import concourse.bass_isa as bass_isa
from concourse.bass_utils import run_bass_kernel_spmd

N_CORES = 8
C = 2048
H = 16
D = 128
HPC = H // N_CORES          # heads per core = 2
PB = 512                    # row panel width
JB = 128                    # key tile width
NEG = -1.0e30
ROPE_BASE = 10000.0

BF = mybir.dt.bfloat16
F16 = mybir.dt.float16
F32 = mybir.dt.float32
F8 = mybir.dt.float8e4
DR = mybir.MatmulPerfMode.DoubleRow
AluAdd = mybir.AluOpType.add
Exp = mybir.ActivationFunctionType.Exp


def build_module(B, T):
    CC = C // 128            # contraction chunks for the projection
    FT = 3 * HPC             # qkv f-tiles per core (q0 q1 k0 k1 v0 v1)
    NPB = T // PB            # panels per batch
    NOC = C // 128           # out-proj column tiles
    NPANEL = B * NPB
    scale = 1.0 / float(np.sqrt(D))

    nc = bacc.Bacc("TRN2", target_bir_lowering=False, debug=False,
                   num_devices=N_CORES)

    # x pre-tiled on host: xt*[g, p, cc*PB + r] = x[g*PB + r, cc*128 + p]
    xt16 = nc.dram_tensor("xt16", [B, 128, CC * PB], BF,
                          kind="ExternalInput").ap()          # pp=0 panels
    xt8 = nc.dram_tensor("xt8", [B * (NPB - 1), 128, CC * PB], F8,
                         kind="ExternalInput").ap()           # pp>=1 panels
    wqkvT16 = nc.dram_tensor("wqkvT16", [FT, 128, CC, 128], BF,
                             kind="ExternalInput").ap()
    wqkvT8 = nc.dram_tensor("wqkvT8", [FT, 128, CC, 128], F8,
                            kind="ExternalInput").ap()
    wprojT = nc.dram_tensor("wprojT", [128, HPC, C], BF,
                            kind="ExternalInput").ap()
    cosT = nc.dram_tensor("cosT", [128, T], BF, kind="ExternalInput").ap()
    sinT = nc.dram_tensor("sinT", [128, T], BF, kind="ExternalInput").ap()
    maskT = nc.dram_tensor("maskT", [128, 896], BF, kind="ExternalInput").ap()
    identT = nc.dram_tensor("identT", [128, 128], BF, kind="ExternalInput").ap()
    permT = nc.dram_tensor("permT", [128, 128], BF, kind="ExternalInput").ap()
    # tiled output: zout[g, p, oc, r] = z[oc*128 + p, g*PB + r]
    zout = nc.dram_tensor("zout", [NPANEL, 128, NOC, PB], BF,
                          kind="ExternalOutput").ap()

    def is16(g):             # bf16 (accurate) panel?
        return g % NPB == 0

    with tile.TileContext(nc) as tc:
        with tc.tile_pool(name="sb", bufs=1) as sb, \
             tc.tile_pool(name="ps", bufs=1, space="PSUM") as ps:
            # ---- resident constants ----
            wq16_sb = sb.tile([128, FT, CC, 128], BF, tag="wq16", bufs=1)
            wq8_sb = sb.tile([128, FT, CC, 128], F8, tag="wq8", bufs=1)
            wproj_sb = sb.tile([128, HPC, C], BF, tag="wproj", bufs=1)
            cos_sb = sb.tile([128, T], BF, tag="cos", bufs=1)
            sin_sb = sb.tile([128, T], BF, tag="sin", bufs=1)
            mask_sb = sb.tile([128, 896], BF, tag="mask", bufs=1)
            ident_sb = sb.tile([128, 128], BF, tag="ident", bufs=1)
            perm_sb = sb.tile([128, 128], BF, tag="perm", bufs=1)

            def load_consts():
                # first-needed-first (wq16 ft0 + x panel 0 are already out);
                # mask/ident feed every diag S-matmul preload -> load early
                nc.scalar.dma_start(out=mask_sb[:], in_=maskT)
                nc.scalar.dma_start(out=ident_sb[:], in_=identT)
                nc.scalar.dma_start(out=perm_sb[:], in_=permT)
                for ft in range(1, FT):
                    q = nc.sync if ft % 2 else nc.scalar
                    q.dma_start(out=wq16_sb[:, ft], in_=wqkvT16[ft])
                nc.scalar.dma_start(out=cos_sb[:], in_=cosT)
                nc.scalar.dma_start(out=sin_sb[:], in_=sinT)
                for ft in range(FT):
                    q = nc.sync if ft % 2 else nc.scalar
                    q.dma_start(out=wq8_sb[:, ft], in_=wqkvT8[ft])
                nc.scalar.dma_start(out=wproj_sb[:], in_=wprojT)

            HC = CC // 2
            xt_map = {}          # panel g -> (xta, xtb)
            bt_map = {}          # batch b -> dict(q=,k=,v=)

            def load_xt(g, split4=False):
                if is16(g):
                    xta = sb.tile([128, HC, PB], BF, tag="xta16", bufs=1,
                                  name=f"xta_{g}")
                    xtb = sb.tile([128, HC, PB], BF, tag="xtb16", bufs=1,
                                  name=f"xtb_{g}")
                    src = xt16[g // NPB].rearrange("p (cc r) -> p cc r", r=PB)
                else:
                    xta = sb.tile([128, HC, PB], F8, tag="xta8", bufs=2,
                                  name=f"xta_{g}")
                    xtb = sb.tile([128, HC, PB], F8, tag="xtb8", bufs=2,
                                  name=f"xtb_{g}")
                    g8 = (g // NPB) * (NPB - 1) + (g % NPB) - 1
                    src = xt8[g8].rearrange("p (cc r) -> p cc r", r=PB)
                # bulk x loads stay OFF the gpsimd channel: rot swaps
                # share it and must not queue behind MB-scale transfers
                if split4:
                    hq = HC // 2
                    nc.sync.dma_start(out=xta[:, :hq], in_=src[:, :hq, :])
                    nc.scalar.dma_start(out=xta[:, hq:],
                                        in_=src[:, hq:HC, :])
                    nc.sync.dma_start(out=xtb[:, :hq],
                                      in_=src[:, HC:HC + hq, :])
                    nc.sync.dma_start(out=xtb[:, hq:], in_=src[:, HC + hq:, :])
                else:
                    nc.sync.dma_start(out=xta[:], in_=src[:, :HC, :])
                    nc.scalar.dma_start(out=xtb[:], in_=src[:, HC:, :])
                xt_map[g] = (xta, xtb)

            def batch_tiles(b):
                if b not in bt_map:
                    bt_map[b] = {
                        "q": [sb.tile([128, T], BF, tag=f"q{h}", bufs=2,
                                      name=f"q{h}_{b}") for h in range(HPC)],
                        "k": [sb.tile([128, T], BF, tag=f"k{h}", bufs=2,
                                      name=f"k{h}_{b}") for h in range(HPC)],
                        "v": [sb.tile([128, T // 128, 128], BF, tag=f"v{h}",
                                      bufs=2, name=f"v{h}_{b}")
                              for h in range(HPC)],
                    }
                return bt_map[b]

            # ---------------- projection (filler stream) ----------------
            proj_state = {}

            def emit_proj_group(b, pp, ft, half):
                g = b * NPB + pp
                st = proj_state.setdefault((b, pp), {"raw": {}, "vst": {},
                                                     "pps": {}})
                if ft == 0 and half == 0:
                    if g + 1 < NPANEL:
                        # batch-0 panels: split across both channels so the
                        # unwoven projection never outruns its x transfers
                        load_xt(g + 1, split4=(g + 1 < NPB))
                xt = xt_map[g]
                if half == 0:
                    pps = ps.tile([128, PB], F32, tag="ps512", bufs=6)
                    st["pps"][ft] = pps
                    ccs = range(0, CC // 2)
                else:
                    pps = st["pps"].pop(ft)
                    ccs = range(CC // 2, CC)
                if is16(g):
                    for cc in ccs:
                        xsrc = (xt[0][:, cc, :] if cc < HC
                                else xt[1][:, cc - HC, :])
                        nc.tensor.matmul(
                            pps[:], lhsT=wq16_sb[:, ft, cc, :], rhs=xsrc,
                            start=(cc == 0), stop=(cc == CC - 1))
                else:
                    for cc in [c for c in ccs if c % 2 == 0]:
                        xsrc = (xt[0][:, cc:cc + 2, :] if cc < HC
                                else xt[1][:, cc - HC:cc - HC + 2, :])
                        nc.tensor.matmul(
                            pps[:], lhsT=wq8_sb[:, ft, cc:cc + 2, :],
                            rhs=xsrc, start=(cc == 0), stop=(cc == CC - 2),
                            perf_mode=DR)
                if half == 0:
                    return
                if ft < 2 * HPC:       # q or k: stage raw for rope
                    raw = sb.tile([128, PB], BF, tag="qkraw", bufs=4,
                                  name=f"raw_{b}_{pp}_{ft}")
                    nc.scalar.copy(out=raw[:], in_=pps[:])
                    st["raw"][ft] = raw
                else:                  # v: stage for dma-transpose
                    vst = sb.tile([128, PB], BF, tag="vstage", bufs=4,
                                  name=f"vst_{b}_{pp}_{ft}")
                    nc.scalar.copy(out=vst[:], in_=pps[:])
                    st["vst"][ft - 2 * HPC] = vst

            def emit_fin_ft(b, pp, ft):
                """RoPE for one q/k tile: DMA rotate-half swap + gpsimd
                muls + one DVE add. Small bursts keep the DVE queue fluid
                for the attention critical path."""
                ts = slice(pp * PB, pp * PB + PB)
                tiles = batch_tiles(b)
                raw = proj_state[(b, pp)]["raw"][ft]
                rotb = sb.tile([128, PB], BF, tag="rotb", bufs=4,
                               name=f"rotb_{b}_{pp}_{ft}")
                if b == 0:
                    # batch 0 is unwoven: the PE is free and the DMA-swap
                    # chain would delay the very first S matmuls
                    rot = ps.tile([128, PB], F32, tag="ps512", bufs=6)
                    nc.tensor.matmul(rot[:], lhsT=perm_sb[:], rhs=raw[:],
                                     start=True, stop=True)
                    nc.scalar.copy(out=rotb[:], in_=rot[:])
                else:
                    nc.gpsimd.dma_start(out=rotb[0:64, :], in_=raw[64:128, :])
                    nc.gpsimd.dma_start(out=rotb[64:128, :],
                                        in_=raw[0:64, :])
                t1 = sb.tile([128, PB], BF, tag="t1", bufs=2)
                nc.vector.tensor_mul(out=t1[:], in0=raw[:], in1=cos_sb[:, ts])
                t2 = sb.tile([128, PB], BF, tag="t2", bufs=2)
                nc.vector.tensor_mul(out=t2[:], in0=rotb[:], in1=sin_sb[:, ts])
                dest = (tiles["q"] if ft < HPC else tiles["k"])[ft % HPC]
                nc.vector.tensor_add(out=dest[:, ts], in0=t1[:], in1=t2[:])

            def emit_fin_v(b, pp):
                tiles = batch_tiles(b)
                st = proj_state.pop((b, pp))
                for h in range(HPC):        # v transposes (DMA xbar)
                    vst = st["vst"][h]
                    for q4 in range(PB // 128):
                        jt = pp * (PB // 128) + q4
                        nc.sync.dma_start_transpose(
                            out=tiles["v"][h][:, jt, :],
                            in_=vst[:, q4 * 128:(q4 + 1) * 128])

            def proj_units(b):
                for pp in range(NPB):
                    for ft in range(FT):
                        yield ("grp", b, pp, ft, 0)
                        yield ("grp", b, pp, ft, 1)
                    for ft in range(2 * HPC):
                        yield ("finft", b, pp, ft)
                    yield ("finv", b, pp)

            def run_proj_unit(u):
                if u[0] == "grp":
                    emit_proj_group(u[1], u[2], u[3], u[4])
                elif u[0] == "finft":
                    emit_fin_ft(u[1], u[2], u[3])
                else:
                    emit_fin_v(u[1], u[2])

            # ---------------- attention (primary stream) ----------------
            def emit_attn_batch(b, filler, op_q):
                tiles = batch_tiles(b)
                q_t, k_t, v_t = tiles["q"], tiles["k"], tiles["v"]

                n_primary = 0
                for pp in range(NPB):
                    n_primary += (pp + 1) * (PB // JB) * HPC + 1
                n_filler = (NPB * (2 * FT + 2 * HPC + 1)
                            if filler is not None else 0)
                frate = 1.05 * n_filler / max(1, n_primary)
                fcredit = 0.0

                def tick():
                    nonlocal fcredit
                    fcredit += frate
                    while fcredit >= 1.0:
                        fcredit -= 1.0
                        u = next(filler, None)
                        if u is not None:
                            run_proj_unit(u)

                # flat cross-panel job list: the S pipeline never drains at
                # panel boundaries (S needs only q/k, no per-panel state)
                jobs = []
                for pp in range(NPB):
                    nj = (pp + 1) * (PB // JB)
                    jobs += [(pp, h, j) for j in range(nj)
                             for h in range(HPC)]

                pctx = {}

                def get_ctx(pp):
                    if pp not in pctx:
                        pctx[pp] = {
                            "ytil": [ps.tile([128, PB], F32, tag="ytil",
                                             bufs=2, name=f"ytil{h}_{b}_{pp}")
                                     for h in range(HPC)],
                            "esum": [[sb.tile([128, PB], F16, tag="esum",
                                              bufs=8,
                                              name=f"esum{h}{par}_{b}_{pp}")
                                      for par in range(2)]
                                     for h in range(HPC)],
                            "blo": [0, 0],
                        }
                    return pctx[pp]

                def emit_S(pp, h, j):
                    kk = j - pp * (PB // JB)
                    lo = max(kk, 0) * 128
                    q0 = pp * PB
                    sps = ps.tile([128, PB], F32, tag="ps512", bufs=6,
                                  name=f"s{h}_{b}_{pp}_{j}")
                    diag = kk >= 0
                    if diag:
                        # causal mask pre-loaded into PSUM via identity
                        # matmul: keeps the mask off the DVE/ACT queues
                        nc.tensor.matmul(
                            sps[:, lo:PB], lhsT=ident_sb[:],
                            rhs=mask_sb[:, 384:384 + (PB - lo)],
                            start=True, stop=False)
                    nc.tensor.matmul(
                        sps[:, lo:PB],
                        lhsT=k_t[h][:, j * JB:(j + 1) * JB],
                        rhs=q_t[h][:, q0 + lo:q0 + PB],
                        start=not diag, stop=True)
                    return sps

                def emit_rest(pp, h, j, sps):
                    ctx = get_ctx(pp)
                    nj = (pp + 1) * (PB // JB)
                    kk = j - pp * (PB // JB)
                    lo = max(kk, 0) * 128
                    e = sb.tile([128, PB], F16, tag="e", bufs=4,
                                name=f"e{h}_{b}_{pp}_{j}")
                    nc.scalar.activation(
                        out=e[:, lo:PB], in_=sps[:, lo:PB],
                        func=Exp, scale=scale)
                    nc.tensor.matmul(ctx["ytil"][h][:, lo:PB],
                                     lhsT=v_t[h][:, j, :],
                                     rhs=e[:, lo:PB], start=(j == 0),
                                     stop=(j == nj - 1))
                    # denominator partial accumulation (DVE, f16 2x)
                    acc = ctx["esum"][h][j % 2]
                    if j < 2:
                        ctx["blo"][j] = lo
                        nc.vector.tensor_copy(out=acc[:, lo:PB],
                                              in_=e[:, lo:PB])
                    else:
                        nc.vector.tensor_tensor(
                            out=acc[:, lo:PB],
                            in0=acc[:, lo:PB], in1=e[:, lo:PB],
                            op=AluAdd)

                def emit_panel_end(pp):
                    # deferred softmax finalize + outproj units for panel pp
                    ctx = pctx.pop(pp)
                    ytil, esum = ctx["ytil"], ctx["esum"]
                    g = b * NPB + pp
                    b0, b1 = ctx["blo"]
                    zbig = sb.tile([128, NOC, PB], BF, tag="zbig", bufs=2,
                                   name=f"zbig_{b}_{pp}")
                    ypair = [sb.tile([128, PB], BF, tag="yp", bufs=4,
                                     name=f"yp{h}_{b}_{pp}")
                             for h in range(HPC)]

                    def mk_den(h, es=esum, yt=ytil, yps=ypair,
                               b0=b0, b1=b1):
                        def emit():
                            ec = sb.tile([128, PB], F16, tag="esumC",
                                         bufs=1)
                            if b0 == b1 == 0:
                                nc.vector.tensor_tensor(
                                    out=ec[:], in0=es[h][0][:],
                                    in1=es[h][1][:], op=AluAdd)
                            else:
                                nc.vector.tensor_copy(
                                    out=ec[:, b0:PB],
                                    in_=es[h][0][:, b0:PB])
                                nc.vector.tensor_tensor(
                                    out=ec[:, b1:PB], in0=ec[:, b1:PB],
                                    in1=es[h][1][:, b1:PB], op=AluAdd)
                            red = sb.tile([128, PB], F32, tag="red",
                                          bufs=2)
                            nc.gpsimd.partition_all_reduce(
                                red[:], ec[:], 128,
                                reduce_op=bass_isa.ReduceOp.add)
                            rec = sb.tile([128, PB], F32, tag="rec",
                                          bufs=2)
                            nc.vector.reciprocal_approx_fast(out=rec[:],
                                                             in_=red[:])
                            nc.vector.tensor_mul(out=yps[h][:],
                                                 in0=yt[h][:], in1=rec[:])
                        return emit

                    def mk_oc(oc, yps=ypair, zbig=zbig):
                        def emit():
                            zps = ps.tile([128, PB], F32, tag="ps512",
                                          bufs=6)
                            for hh in range(HPC):
                                nc.tensor.matmul(
                                    zps[:],
                                    lhsT=wproj_sb[:, hh,
                                                  oc * 128:(oc + 1) * 128],
                                    rhs=yps[hh][:],
                                    start=(hh == 0), stop=(hh == HPC - 1))
                            if oc % 2:
                                nc.vector.tensor_copy(out=zbig[:, oc, :],
                                                      in_=zps[:])
                            else:
                                nc.scalar.copy(out=zbig[:, oc, :],
                                               in_=zps[:])
                        return emit

                    def mk_dma(g=g, zbig=zbig, quarter=0):
                        # quarter DMAs spread across all three channels
                        def emit():
                            qs = slice(quarter * (NOC // 4),
                                       (quarter + 1) * (NOC // 4))
                            eng = [nc.sync, nc.scalar, nc.gpsimd,
                                   nc.gpsimd][quarter]
                            eng.dma_start(out=zout[g][:, qs],
                                          in_=zbig[:, qs])
                        return emit

                    # den units MUST drain early (release ytil PSUM banks)
                    op_q[0:0] = [mk_den(h) for h in range(HPC)]
                    for oc in range(NOC):
                        op_q.append(mk_oc(oc))
                        if oc % (NOC // 4) == NOC // 4 - 1:
                            op_q.append(mk_dma(quarter=oc // (NOC // 4)))

                spss = {}
                for idx in range(min(2, len(jobs))):
                    spss[jobs[idx]] = emit_S(*jobs[idx])
                for idx, (pp, h, j) in enumerate(jobs):
                    get_ctx(pp)
                    if op_q:
                        op_q.pop(0)()
                    if op_q and len(op_q) > 20:
                        op_q.pop(0)()
                    if idx + 2 < len(jobs):
                        spss[jobs[idx + 2]] = emit_S(*jobs[idx + 2])
                    emit_rest(pp, h, j, spss.pop((pp, h, j)))
                    tick()
                    if (idx + 1 == len(jobs)
                            or jobs[idx + 1][0] != pp):
                        emit_panel_end(pp)

                # flush filler at end of batch slot
                while True:
                    u = next(filler, None) if filler is not None else None
                    if u is None:
                        break
                    run_proj_unit(u)

            # ---------------- top-level schedule ----------------
            nc.scalar.dma_start(out=wq16_sb[:, 0], in_=wqkvT16[0])
            load_xt(0, split4=True)
            load_consts()
            for u in proj_units(0):       # batch 0 projection, unwoven
                run_proj_unit(u)
            op_q = []
            for b in range(B):
                filler = iter(proj_units(b + 1)) if b + 1 < B else None
                emit_attn_batch(b, filler, op_q)
            while op_q:                   # tail outproj units
                op_q.pop(0)()

    nc.compile()
    return nc


_module_cache = {}


def _get_module(B, T):
    key = (B, T)
    if key not in _module_cache:
        _module_cache[key] = build_module(B, T)
    return _module_cache[key]


def _host_prep(x, Wqkv, Wproj, B, T):
    bf16 = ml_dtypes.bfloat16
    f8 = ml_dtypes.float8_e4m3
    NPB = T // PB
    CC = C // 128
    # [B, NPB, PB, CC, 128] -> per panel [128, CC, PB]
    x5 = x.reshape(B, NPB, PB, CC, 128).transpose(0, 1, 4, 3, 2)
    xt16 = np.ascontiguousarray(
        x5[:, 0].reshape(B, 128, CC * PB)).astype(bf16)
    xt8 = np.ascontiguousarray(
        x5[:, 1:].reshape(B * (NPB - 1), 128, CC * PB)).astype(bf16).astype(f8)

    inv = 1.0 / (ROPE_BASE ** (np.arange(0, D, 2, dtype=np.float32) / D))
    t = np.arange(T, dtype=np.float32)
    fr = np.outer(t, inv)                      # [T, 64]
    emb = np.concatenate([fr, fr], -1)         # [T, 128]
    cosT = np.ascontiguousarray(np.cos(emb).T).astype(bf16)
    # rotate-half sign folded into the sin table (swap * sign * sin)
    sgn = np.where(np.arange(D)[:, None] < 64, -1.0, 1.0).astype(np.float32)
    sinT = np.ascontiguousarray(np.sin(emb).T * sgn).astype(bf16)

    g = np.arange(896)[None, :]
    p = np.arange(128)[:, None]
    maskT = np.where(g >= p + 384, 0.0, NEG).astype(np.float32).astype(bf16)
    identT = np.eye(128, dtype=np.float32).astype(bf16)
    permT = np.zeros((128, 128), np.float32)
    for j in range(64):
        permT[j + 64, j] = 1.0
        permT[j, j + 64] = 1.0
    permT = permT.astype(bf16)

    in_maps = []
    for c in range(N_CORES):
        heads = [HPC * c + h for h in range(HPC)]
        rows = []
        for blk in range(3):                   # q, k, v blocks of Wqkv
            for h in heads:
                r0 = blk * C + h * D
                rows.append(Wqkv[r0:r0 + D])
        wslice = np.concatenate(rows, 0)       # [FT*128, C]
        wq16 = np.ascontiguousarray(
            wslice.T.reshape(C // 128, 128, 3 * HPC, 128).transpose(2, 1, 0, 3)
        ).astype(bf16)
        wq8 = wq16.astype(f8)
        cols = np.concatenate([np.arange(h * D, (h + 1) * D) for h in heads])
        wprojT = np.ascontiguousarray(
            Wproj[:, cols].T.reshape(len(heads), 128, C).transpose(1, 0, 2)
        ).astype(bf16)
        in_maps.append({
            "xt16": xt16,
            "xt8": xt8,
            "wqkvT16": wq16,
            "wqkvT8": wq8,
            "wprojT": wprojT,
            "cosT": cosT,
            "sinT": sinT,
            "maskT": maskT,
            "identT": identT,
            "permT": permT,
        })
    return in_maps


last_results = None


def kernel(x, Wqkv, Wproj, _trace=False, _trace_kwargs=None):
    global last_results
    x = np.asarray(x, dtype=np.float32)
    Wqkv = np.asarray(Wqkv, dtype=np.float32)
    Wproj = np.asarray(Wproj, dtype=np.float32)
    B, T, _C = x.shape
    assert _C == C and T % PB == 0

    nc = _get_module(B, T)
    in_maps = _host_prep(x, Wqkv, Wproj, B, T)
    res = run_bass_kernel_spmd(nc, in_maps, core_ids=list(range(N_CORES)),
                               trace=_trace, **(_trace_kwargs or {}))
    last_results = res
    z = res.results[0]["zout"].astype(np.float32)
    for c in range(1, N_CORES):
        z += res.results[c]["zout"].astype(np.float32)
    # zout[g, p, oc, r] = z[oc*128+p, g*PB+r];  y[t, c] = z[c, t]
    y = z.transpose(0, 3, 2, 1).reshape(B, T, C)
    return y


# revision 43
# speedup vs baseline: 1.1516x; 1.1516x over previous
"""Causal self-attention (QKV proj + RoPE + causal SDPA + out proj) on 8 trn2 cores.

Sharding: tensor-parallel over heads. Each core owns 2 of 16 heads:
  - Wqkv column-split (the core's q/k/v head rows), Wproj row-split.
  - Each core computes a full-shape partial of the output projection;
    the 8 partials are summed (and transposed back) on the host.

Device-side layout: everything runs transposed (x^T fed as [C, B*T];
qkv^T = W @ x keeps head dims on partitions). v is transposed on-chip
with DMA xbar transposes.

Precision: query panels 512.. (pp>=1) run the QKV projection in
fp8-e4m3 DoubleRow (2 k-tiles per PE pass, 2x matmul rate); panel 0 of
each batch stays bf16 because short-context rows lack softmax error
attenuation. Attention itself stays bf16/f16.

PE offloads vs the all-bf16 version: rotate-half runs as SBUF->SBUF
DMA partition swaps (gpsimd queue) instead of PE permute-matmuls; the
softmax denominator broadcast matmul is deferred one job behind its
scalar copy so it never stalls the in-order PE queue.

Schedule: batch b's attention+outproj is woven with batch b+1's
projection so the tensor engine never starves on the exp (scalar
engine) dependency chain.
"""
import sys

sys.path.insert(0, "/opt/trn_rl_repo")

import numpy as np
import ml_dtypes

import concourse.bacc as bacc
import concourse.mybir as mybir
import concourse.tile as tile
import concourse.bass_isa as bass_isa
from concourse.bass_utils import run_bass_kernel_spmd

N_CORES = 8
C = 2048
H = 16
D = 128
HPC = H // N_CORES          # heads per core = 2
PB = 512                    # row panel width
JB = 128                    # key tile width
NEG = -1.0e30
ROPE_BASE = 10000.0

BF = mybir.dt.bfloat16
F16 = mybir.dt.float16
F32 = mybir.dt.float32
F8 = mybir.dt.float8e4
DR = mybir.MatmulPerfMode.DoubleRow
AluAdd = mybir.AluOpType.add
Exp = mybir.ActivationFunctionType.Exp


def build_module(B, T):
    CC = C // 128            # contraction chunks for the projection
    FT = 3 * HPC             # qkv f-tiles per core (q0 q1 k0 k1 v0 v1)
    NPB = T // PB            # panels per batch
    NOC = C // 128           # out-proj column tiles
    NPANEL = B * NPB
    scale = 1.0 / float(np.sqrt(D))

    nc = bacc.Bacc("TRN2", target_bir_lowering=False, debug=False,
                   num_devices=N_CORES)

    # x pre-tiled on host: xt*[g, p, cc*PB + r] = x[g*PB + r, cc*128 + p]
    xt16 = nc.dram_tensor("xt16", [B, 128, CC * PB], BF,
                          kind="ExternalInput").ap()          # pp=0 panels
    xt8 = nc.dram_tensor("xt8", [B * (NPB - 1), 128, CC * PB], F8,
                         kind="ExternalInput").ap()           # pp>=1 panels
    wqkvT16 = nc.dram_tensor("wqkvT16", [FT, 128, CC, 128], BF,
                             kind="ExternalInput").ap()
    wqkvT8 = nc.dram_tensor("wqkvT8", [FT, 128, CC, 128], F8,
                            kind="ExternalInput").ap()
    wprojT = nc.dram_tensor("wprojT", [128, HPC, C], BF,
                            kind="ExternalInput").ap()
    cosT = nc.dram_tensor("cosT", [128, T], BF, kind="ExternalInput").ap()
    sinT = nc.dram_tensor("sinT", [128, T], BF, kind="ExternalInput").ap()
    maskT = nc.dram_tensor("maskT", [128, 896], BF, kind="ExternalInput").ap()
    identT = nc.dram_tensor("identT", [128, 128], BF, kind="ExternalInput").ap()
    permT = nc.dram_tensor("permT", [128, 128], BF, kind="ExternalInput").ap()
    # tiled output: zout[g, p, oc, r] = z[oc*128 + p, g*PB + r]
    zout = nc.dram_tensor("zout", [NPANEL, 128, NOC, PB], BF,
                          kind="ExternalOutput").ap()

    def is16(g):             # bf16 (accurate) panel?
        return g % NPB == 0

    with tile.TileContext(nc) as tc:
        with tc.tile_pool(name="sb", bufs=1) as sb, \
             tc.tile_pool(name="ps", bufs=1, space="PSUM") as ps:
            # ---- resident constants ----
            wq16_sb = sb.tile([128, FT, CC, 128], BF, tag="wq16", bufs=1)
            wq8_sb = sb.tile([128, FT, CC, 128], F8, tag="wq8", bufs=1)
            wproj_sb = sb.tile([128, HPC, C], BF, tag="wproj", bufs=1)
            cos_sb = sb.tile([128, T], BF, tag="cos", bufs=1)
            sin_sb = sb.tile([128, T], BF, tag="sin", bufs=1)
            mask_sb = sb.tile([128, 896], BF, tag="mask", bufs=1)
            ident_sb = sb.tile([128, 128], BF, tag="ident", bufs=1)
            perm_sb = sb.tile([128, 128], BF, tag="perm", bufs=1)

            def load_consts():
                # first-needed-first (wq16 ft0 + x panel 0 are already out);
                # mask/ident feed every diag S-matmul preload -> load early
                nc.scalar.dma_start(out=mask_sb[:], in_=maskT)
                nc.scalar.dma_start(out=ident_sb[:], in_=identT)
                nc.scalar.dma_start(out=perm_sb[:], in_=permT)
                for ft in range(1, FT):
                    q = nc.sync if ft % 2 else nc.scalar
                    q.dma_start(out=wq16_sb[:, ft], in_=wqkvT16[ft])
                nc.scalar.dma_start(out=cos_sb[:], in_=cosT)
                nc.scalar.dma_start(out=sin_sb[:], in_=sinT)
                for ft in range(FT):
                    q = nc.sync if ft % 2 else nc.scalar
                    q.dma_start(out=wq8_sb[:, ft], in_=wqkvT8[ft])
                nc.scalar.dma_start(out=wproj_sb[:], in_=wprojT)

            HC = CC // 2
            xt_map = {}          # panel g -> (xta, xtb)
            bt_map = {}          # batch b -> dict(q=,k=,v=)

            def load_xt(g, split4=False):
                if is16(g):
                    xta = sb.tile([128, HC, PB], BF, tag="xta16", bufs=1,
                                  name=f"xta_{g}")
                    xtb = sb.tile([128, HC, PB], BF, tag="xtb16", bufs=1,
                                  name=f"xtb_{g}")
                    src = xt16[g // NPB].rearrange("p (cc r) -> p cc r", r=PB)
                else:
                    xta = sb.tile([128, HC, PB], F8, tag="xta8", bufs=2,
                                  name=f"xta_{g}")
                    xtb = sb.tile([128, HC, PB], F8, tag="xtb8", bufs=2,
                                  name=f"xtb_{g}")
                    g8 = (g // NPB) * (NPB - 1) + (g % NPB) - 1
                    src = xt8[g8].rearrange("p (cc r) -> p cc r", r=PB)
                # bulk x loads stay OFF the gpsimd channel: rot swaps
                # share it and must not queue behind MB-scale transfers
                if split4:
                    hq = HC // 2
                    nc.sync.dma_start(out=xta[:, :hq], in_=src[:, :hq, :])
                    nc.scalar.dma_start(out=xta[:, hq:],
                                        in_=src[:, hq:HC, :])
                    nc.sync.dma_start(out=xtb[:, :hq],
                                      in_=src[:, HC:HC + hq, :])
                    nc.sync.dma_start(out=xtb[:, hq:], in_=src[:, HC + hq:, :])
                else:
                    nc.sync.dma_start(out=xta[:], in_=src[:, :HC, :])
                    nc.scalar.dma_start(out=xtb[:], in_=src[:, HC:, :])
                xt_map[g] = (xta, xtb)

            def batch_tiles(b):
                if b not in bt_map:
                    bt_map[b] = {
                        "q": [sb.tile([128, T], BF, tag=f"q{h}", bufs=2,
                                      name=f"q{h}_{b}") for h in range(HPC)],
                        "k": [sb.tile([128, T], BF, tag=f"k{h}", bufs=2,
                                      name=f"k{h}_{b}") for h in range(HPC)],
                        "v": [sb.tile([128, T // 128, 128], BF, tag=f"v{h}",
                                      bufs=2, name=f"v{h}_{b}")
                              for h in range(HPC)],
                    }
                return bt_map[b]

            # ---------------- projection (filler stream) ----------------
            proj_state = {}

            def emit_proj_group(b, pp, ft, half):
                g = b * NPB + pp
                st = proj_state.setdefault((b, pp), {"raw": {}, "vst": {},
                                                     "pps": {}})
                if ft == 0 and half == 0:
                    if g + 1 < NPANEL:
                        # batch-0 panels: split across both channels so the
                        # unwoven projection never outruns its x transfers
                        load_xt(g + 1, split4=(g + 1 < NPB))
                xt = xt_map[g]
                if half == 0:
                    pps = ps.tile([128, PB], F32, tag="ps512", bufs=6)
                    st["pps"][ft] = pps
                    ccs = range(0, CC // 2)
                else:
                    pps = st["pps"].pop(ft)
                    ccs = range(CC // 2, CC)
                if is16(g):
                    for cc in ccs:
                        xsrc = (xt[0][:, cc, :] if cc < HC
                                else xt[1][:, cc - HC, :])
                        nc.tensor.matmul(
                            pps[:], lhsT=wq16_sb[:, ft, cc, :], rhs=xsrc,
                            start=(cc == 0), stop=(cc == CC - 1))
                else:
                    for cc in [c for c in ccs if c % 2 == 0]:
                        xsrc = (xt[0][:, cc:cc + 2, :] if cc < HC
                                else xt[1][:, cc - HC:cc - HC + 2, :])
                        nc.tensor.matmul(
                            pps[:], lhsT=wq8_sb[:, ft, cc:cc + 2, :],
                            rhs=xsrc, start=(cc == 0), stop=(cc == CC - 2),
                            perf_mode=DR)
                if half == 0:
                    return
                if ft < 2 * HPC:       # q or k: stage raw for rope
                    raw = sb.tile([128, PB], BF, tag="qkraw", bufs=4,
                                  name=f"raw_{b}_{pp}_{ft}")
                    nc.scalar.copy(out=raw[:], in_=pps[:])
                    st["raw"][ft] = raw
                else:                  # v: stage for dma-transpose
                    vst = sb.tile([128, PB], BF, tag="vstage", bufs=4,
                                  name=f"vst_{b}_{pp}_{ft}")
                    nc.scalar.copy(out=vst[:], in_=pps[:])
                    st["vst"][ft - 2 * HPC] = vst

            def emit_fin_ft(b, pp, ft):
                """RoPE for one q/k tile: DMA rotate-half swap + gpsimd
                muls + one DVE add. Small bursts keep the DVE queue fluid
                for the attention critical path."""
                ts = slice(pp * PB, pp * PB + PB)
                tiles = batch_tiles(b)
                raw = proj_state[(b, pp)]["raw"][ft]
                rotb = sb.tile([128, PB], BF, tag="rotb", bufs=4,
                               name=f"rotb_{b}_{pp}_{ft}")
                if b == 0:
                    # batch 0 is unwoven: the PE is free and the DMA-swap
                    # chain would delay the very first S matmuls
                    rot = ps.tile([128, PB], F32, tag="ps512", bufs=6)
                    nc.tensor.matmul(rot[:], lhsT=perm_sb[:], rhs=raw[:],
                                     start=True, stop=True)
                    nc.scalar.copy(out=rotb[:], in_=rot[:])
                else:
                    nc.gpsimd.dma_start(out=rotb[0:64, :], in_=raw[64:128, :])
                    nc.gpsimd.dma_start(out=rotb[64:128, :],
                                        in_=raw[0:64, :])
                t1 = sb.tile([128, PB], BF, tag="t1", bufs=2)
                nc.vector.tensor_mul(out=t1[:], in0=raw[:], in1=cos_sb[:, ts])
                t2 = sb.tile([128, PB], BF, tag="t2", bufs=2)
                nc.vector.tensor_mul(out=t2[:], in0=rotb[:], in1=sin_sb[:, ts])
                dest = (tiles["q"] if ft < HPC else tiles["k"])[ft % HPC]
                nc.vector.tensor_add(out=dest[:, ts], in0=t1[:], in1=t2[:])

            def emit_fin_v(b, pp):
                tiles = batch_tiles(b)
                st = proj_state.pop((b, pp))
                for h in range(HPC):        # v transposes (DMA xbar)
                    vst = st["vst"][h]
                    for q4 in range(PB // 128):
                        jt = pp * (PB // 128) + q4
                        nc.sync.dma_start_transpose(
                            out=tiles["v"][h][:, jt, :],
                            in_=vst[:, q4 * 128:(q4 + 1) * 128])

            def proj_units(b):
                for pp in range(NPB):
                    for ft in range(FT):
                        yield ("grp", b, pp, ft, 0)
                        yield ("grp", b, pp, ft, 1)
                    for ft in range(2 * HPC):
                        yield ("finft", b, pp, ft)
                    yield ("finv", b, pp)

            def run_proj_unit(u):
                if u[0] == "grp":
                    emit_proj_group(u[1], u[2], u[3], u[4])
                elif u[0] == "finft":
                    emit_fin_ft(u[1], u[2], u[3])
                else:
                    emit_fin_v(u[1], u[2])

            # ---------------- attention (primary stream) ----------------
            def emit_attn_batch(b, filler, op_q):
                tiles = batch_tiles(b)
                q_t, k_t, v_t = tiles["q"], tiles["k"], tiles["v"]

                n_primary = 0
                for pp in range(NPB):
                    n_primary += (pp + 1) * (PB // JB) * HPC + 1
                n_filler = (NPB * (2 * FT + 2 * HPC + 1)
                            if filler is not None else 0)
                frate = 1.05 * n_filler / max(1, n_primary)
                fcredit = 0.0

                def tick():
                    nonlocal fcredit
                    fcredit += frate
                    while fcredit >= 1.0:
                        fcredit -= 1.0
                        u = next(filler, None)
                        if u is not None:
                            run_proj_unit(u)

                for pp in range(NPB):
                    nj = (pp + 1) * (PB // JB)
                    q0 = pp * PB
                    ytil = [ps.tile([128, PB], F32, tag="ytil", bufs=2,
                                    name=f"ytil{h}_{b}_{pp}")
                            for h in range(HPC)]
                    esum = [[sb.tile([128, PB], F16, tag="esum", bufs=8,
                                     name=f"esum{h}{par}_{b}_{pp}")
                             for par in range(2)] for h in range(HPC)]
                    blo = [0, 0]   # valid-from column of partial 0/1

                    def emit_S(h, j):
                        kk = j - pp * (PB // JB)
                        lo = max(kk, 0) * 128
                        sps = ps.tile([128, PB], F32, tag="ps512", bufs=6,
                                      name=f"s{h}_{b}_{pp}_{j}")
                        diag = kk >= 0
                        if diag:
                            # causal mask pre-loaded into PSUM via identity
                            # matmul: keeps the mask off the DVE/ACT queues
                            nc.tensor.matmul(
                                sps[:, lo:PB], lhsT=ident_sb[:],
                                rhs=mask_sb[:, 384:384 + (PB - lo)],
                                start=True, stop=False)
                        nc.tensor.matmul(
                            sps[:, lo:PB],
                            lhsT=k_t[h][:, j * JB:(j + 1) * JB],
                            rhs=q_t[h][:, q0 + lo:q0 + PB],
                            start=not diag, stop=True)
                        return sps

                    def emit_rest(h, j, sps):
                        kk = j - pp * (PB // JB)
                        lo = max(kk, 0) * 128
                        e = sb.tile([128, PB], F16, tag="e", bufs=4,
                                    name=f"e{h}_{b}_{pp}_{j}")
                        nc.scalar.activation(
                            out=e[:, lo:PB], in_=sps[:, lo:PB],
                            func=Exp, scale=scale)
                        nc.tensor.matmul(ytil[h][:, lo:PB],
                                         lhsT=v_t[h][:, j, :],
                                         rhs=e[:, lo:PB], start=(j == 0),
                                         stop=(j == nj - 1))
                        # denominator partial accumulation (DVE, f16 2x)
                        acc = esum[h][j % 2]
                        if j < 2:
                            blo[j] = lo
                            nc.vector.tensor_copy(out=acc[:, lo:PB],
                                                  in_=e[:, lo:PB])
                        else:
                            nc.vector.tensor_tensor(
                                out=acc[:, lo:PB],
                                in0=acc[:, lo:PB], in1=e[:, lo:PB],
                                op=AluAdd)

                    # h-interleaved: two independent S->exp->AV chains in
                    # flight so one exp round-trip hides behind the other
                    jobs = [(h, j) for j in range(nj) for h in range(HPC)]
                    spss = {}
                    for idx in range(min(2, len(jobs))):
                        spss[jobs[idx]] = emit_S(*jobs[idx])
                    for idx, (h, j) in enumerate(jobs):
                        if op_q:
                            op_q.pop(0)()
                        if op_q and len(op_q) > 20:
                            op_q.pop(0)()
                        if idx + 2 < len(jobs):
                            spss[jobs[idx + 2]] = emit_S(*jobs[idx + 2])
                        emit_rest(h, j, spss.pop((h, j)))
                        tick()

                    # ---- deferred softmax finalize + outproj units ----
                    g = b * NPB + pp
                    b0, b1 = blo[0], blo[1]
                    zbig = sb.tile([128, NOC, PB], BF, tag="zbig", bufs=2,
                                   name=f"zbig_{b}_{pp}")
                    ypair = [sb.tile([128, PB], BF, tag="yp", bufs=4,
                                     name=f"yp{h}_{b}_{pp}")
                             for h in range(HPC)]

                    def mk_den(h, es=esum, yt=ytil, yps=ypair, b0=b0, b1=b1):
                        def emit():
                            ec = sb.tile([128, PB], F16, tag="esumC", bufs=1)
                            if b0 == b1 == 0:
                                nc.vector.tensor_tensor(
                                    out=ec[:], in0=es[h][0][:],
                                    in1=es[h][1][:], op=AluAdd)
                            else:
                                nc.vector.tensor_copy(out=ec[:, b0:PB],
                                                      in_=es[h][0][:, b0:PB])
                                nc.vector.tensor_tensor(
                                    out=ec[:, b1:PB], in0=ec[:, b1:PB],
                                    in1=es[h][1][:, b1:PB], op=AluAdd)
                            red = sb.tile([128, PB], F32, tag="red", bufs=2)
                            nc.gpsimd.partition_all_reduce(
                                red[:], ec[:], 128,
                                reduce_op=bass_isa.ReduceOp.add)
                            rec = sb.tile([128, PB], F32, tag="rec", bufs=2)
                            nc.vector.reciprocal_approx_fast(out=rec[:],
                                                             in_=red[:])
                            nc.vector.tensor_mul(out=yps[h][:],
                                                 in0=yt[h][:], in1=rec[:])
                        return emit

                    def mk_oc(oc, yps=ypair, zbig=zbig):
                        def emit():
                            zps = ps.tile([128, PB], F32, tag="ps512", bufs=6)
                            for hh in range(HPC):
                                nc.tensor.matmul(
                                    zps[:],
                                    lhsT=wproj_sb[:, hh,
                                                  oc * 128:(oc + 1) * 128],
                                    rhs=yps[hh][:],
                                    start=(hh == 0), stop=(hh == HPC - 1))
                            if oc % 2:
                                nc.vector.tensor_copy(out=zbig[:, oc, :],
                                                      in_=zps[:])
                            else:
                                nc.scalar.copy(out=zbig[:, oc, :],
                                               in_=zps[:])
                        return emit

                    def mk_dma(g=g, zbig=zbig, quarter=0):
                        # quarter DMAs across sync+gpsimd channels: bulk
                        # output never delays a latency-critical transfer
                        # by more than ~0.5 MB
                        def emit():
                            qs = slice(quarter * (NOC // 4),
                                       (quarter + 1) * (NOC // 4))
                            eng = nc.sync if quarter < 2 else nc.gpsimd
                            eng.dma_start(out=zout[g][:, qs],
                                          in_=zbig[:, qs])
                        return emit

                    # den units MUST drain early (release ytil PSUM banks)
                    op_q[0:0] = [mk_den(h) for h in range(HPC)]
                    for oc in range(NOC):
                        op_q.append(mk_oc(oc))
                        if oc % (NOC // 4) == NOC // 4 - 1:
                            op_q.append(mk_dma(quarter=oc // (NOC // 4)))

                # flush filler at end of batch slot
                while True:
                    u = next(filler, None) if filler is not None else None
                    if u is None:
                        break
                    run_proj_unit(u)

            # ---------------- top-level schedule ----------------
            nc.scalar.dma_start(out=wq16_sb[:, 0], in_=wqkvT16[0])
            load_xt(0, split4=True)
            load_consts()
            for u in proj_units(0):       # batch 0 projection, unwoven
                run_proj_unit(u)
            op_q = []
            for b in range(B):
                filler = iter(proj_units(b + 1)) if b + 1 < B else None
                emit_attn_batch(b, filler, op_q)
            while op_q:                   # tail outproj units
                op_q.pop(0)()

    nc.compile()
    return nc


_module_cache = {}


def _get_module(B, T):
    key = (B, T)
    if key not in _module_cache:
        _module_cache[key] = build_module(B, T)
    return _module_cache[key]


def _host_prep(x, Wqkv, Wproj, B, T):
    bf16 = ml_dtypes.bfloat16
    f8 = ml_dtypes.float8_e4m3
    NPB = T // PB
    CC = C // 128
    # [B, NPB, PB, CC, 128] -> per panel [128, CC, PB]
    x5 = x.reshape(B, NPB, PB, CC, 128).transpose(0, 1, 4, 3, 2)
    xt16 = np.ascontiguousarray(
        x5[:, 0].reshape(B, 128, CC * PB)).astype(bf16)
    xt8 = np.ascontiguousarray(
        x5[:, 1:].reshape(B * (NPB - 1), 128, CC * PB)).astype(bf16).astype(f8)

    inv = 1.0 / (ROPE_BASE ** (np.arange(0, D, 2, dtype=np.float32) / D))
    t = np.arange(T, dtype=np.float32)
    fr = np.outer(t, inv)                      # [T, 64]
    emb = np.concatenate([fr, fr], -1)         # [T, 128]
    cosT = np.ascontiguousarray(np.cos(emb).T).astype(bf16)
    # rotate-half sign folded into the sin table (swap * sign * sin)
    sgn = np.where(np.arange(D)[:, None] < 64, -1.0, 1.0).astype(np.float32)
    sinT = np.ascontiguousarray(np.sin(emb).T * sgn).astype(bf16)

    g = np.arange(896)[None, :]
    p = np.arange(128)[:, None]
    maskT = np.where(g >= p + 384, 0.0, NEG).astype(np.float32).astype(bf16)
    identT = np.eye(128, dtype=np.float32).astype(bf16)
    permT = np.zeros((128, 128), np.float32)
    for j in range(64):
        permT[j + 64, j] = 1.0
        permT[j, j + 64] = 1.0
    permT = permT.astype(bf16)

    in_maps = []
    for c in range(N_CORES):
        heads = [HPC * c + h for h in range(HPC)]
        rows = []
        for blk in range(3):                   # q, k, v blocks of Wqkv
            for h in heads:
                r0 = blk * C + h * D
                rows.append(Wqkv[r0:r0 + D])
        wslice = np.concatenate(rows, 0)       # [FT*128, C]
        wq16 = np.ascontiguousarray(
            wslice.T.reshape(C // 128, 128, 3 * HPC, 128).transpose(2, 1, 0, 3)
        ).astype(bf16)
        wq8 = wq16.astype(f8)
        cols = np.concatenate([np.arange(h * D, (h + 1) * D) for h in heads])
        wprojT = np.ascontiguousarray(
            Wproj[:, cols].T.reshape(len(heads), 128, C).transpose(1, 0, 2)
        ).astype(bf16)
        in_maps.append({
            "xt16": xt16,
            "xt8": xt8,
            "wqkvT16": wq16,
            "wqkvT8": wq8,
            "wprojT": wprojT,
            "cosT": cosT,
            "sinT": sinT,
            "maskT": maskT,
            "identT": identT,
            "permT": permT,
        })
    return in_maps


last_results = None


def kernel(x, Wqkv, Wproj, _trace=False, _trace_kwargs=None):
    global last_results
    x = np.asarray(x, dtype=np.float32)
    Wqkv = np.asarray(Wqkv, dtype=np.float32)
    Wproj = np.asarray(Wproj, dtype=np.float32)
    B, T, _C = x.shape
    assert _C == C and T % PB == 0

    nc = _get_module(B, T)
    in_maps = _host_prep(x, Wqkv, Wproj, B, T)
    res = run_bass_kernel_spmd(nc, in_maps, core_ids=list(range(N_CORES)),
                               trace=_trace, **(_trace_kwargs or {}))
    last_results = res
    z = res.results[0]["zout"].astype(np.float32)
    for c in range(1, N_CORES):
        z += res.results[c]["zout"].astype(np.float32)
    # zout[g, p, oc, r] = z[oc*128+p, g*PB+r];  y[t, c] = z[c, t]
    y = z.transpose(0, 3, 2, 1).reshape(B, T, C)
    return y
